# revision 6
# baseline (speedup 1.0000x reference)
"""Trainium2 Bass kernel for nn_EquivariantDecoder (B=1, N=512, LAT=128, H=256,
NL=3, NH=4).

Strategy (8 NeuronCores, SPMD):
  - The transformer trunk (input proj + 3 encoder layers) is replicated on all
    cores: activations are kept feature-major (hT = [H, N] tiles) so every
    linear is a single PE pass with host-pretransposed fp16 weights; fp32 PSUM.
  - Attention uses a transposed-softmax formulation: scoresT[k, q] tiles come
    out of PE directly, Exp on ACT (scores are tiny, no max subtraction
    needed), and the softmax denominator rides along the O-matmul as an extra
    ones-column of V (psum row 64).  No PE transposes anywhere.
  - LayerNorm runs feature-major: per-token sum/sumsq via ones-column matmuls,
    rsqrt as Exp(-0.5*Ln(v)) (keeps every ACT func in one table set), and the
    row stats broadcast back over partitions with K=1 ones-row matmuls.
  - The N x N distance head is sharded by rows: core k computes rows
    [64k, 64k+64).  Row selection is data-driven (a per-core one-hot `sel`
    matrix), so all cores run an identical program.  pair1 = silu(ai + aj) is
    built with per-partition-scalar adds (DVE/Pool) + big batched ACT silus;
    pair2 via PE; the final w3 dot runs pair2-chunk-stationary on PE with N=2,
    producing dist columns gathered by tiny DVE copies.  softplus = Ln(1+Exp).
  - Host: embeds atom types, transposes/casts weights to fp16, fans out to the
    8 cores, reassembles and symmetrizes dist.
"""
import numpy as np

import concourse.bass as bass
import concourse.tile as tile
from concourse import bacc, mybir
from concourse.bass_utils import run_bass_kernel_spmd

F32 = mybir.dt.float32
F16 = mybir.dt.float16
AF = mybir.ActivationFunctionType
OP = mybir.AluOpType

N = 512
LAT = 128
EMB = 64
ZIN = LAT + EMB          # 192
H = 256
NH = 4
DH = 64
NL = 3
FF = 4 * H               # 1024
NCORES = 8
R = N // NCORES          # 64 dist rows per core
LN_EPS = 1e-5

_BUILD_CACHE = {}


def _build():
    nc = bacc.Bacc("TRN2", target_bir_lowering=False)

    def din(name, shape, dt=F16):
        return nc.dram_tensor(name, list(shape), dt, kind="ExternalInput")

    # ---- dram inputs ----
    z_inT_d = din("z_inT", [ZIN, N])
    zg_d = din("zg", [LAT, 2])
    ipw1_d = din("ipw1", [ZIN, H])
    ipb1_d = din("ipb1", [H, 1], F32)
    ipw2_d = din("ipw2", [H, H])
    ipb2_d = din("ipb2", [H, 1], F32)
    gpw1_d = din("gpw1", [LAT, H])
    gpb1_d = din("gpb1", [H, 1], F32)
    gpw2_d = din("gpw2", [H, H])
    gpb2_d = din("gpb2", [H, 1], F32)
    wqk_d = din("wqk", [NL, H, 2 * H])
    bqk_d = din("bqk", [NL, 2 * H, 1], F32)
    wv_d = din("wv", [NL, H, H])
    bv_d = din("bv", [NL, 1, H])
    wo_d = din("wo", [NL, H, H])
    bwo_d = din("bwo", [NL, H, 1], F32)
    ln1s_d = din("ln1s", [NL, H, 1], F32)
    ln1b_d = din("ln1b", [NL, H, 1], F32)
    w1_d = din("w1", [NL, H, FF])
    b1_d = din("b1", [NL, FF, 1], F32)
    w2_d = din("w2", [NL, FF, H])
    b2_d = din("b2", [NL, H, 1], F32)
    ln2s_d = din("ln2s", [NL, H, 1], F32)
    ln2b_d = din("ln2b", [NL, H, 1], F32)
    dpw1a_d = din("dpw1a", [H, H])
    dpw1b_d = din("dpw1b", [H, H])
    dpb1_d = din("dpb1", [H, 1], F32)
    dpw2_d = din("dpw2", [H, H])
    dpb2_d = din("dpb2", [H, 1], F32)
    dpw3_d = din("dpw3", [128, 4])          # [kt-part, dup cols (kt0,kt0,kt1,kt1)]
    dpb3_d = din("dpb3", [128, 1], F32)     # b3 broadcast over partitions
    cpw1_d = din("cpw1", [H, H])
    cpb1_d = din("cpb1", [H, 1], F32)
    cpw2_d = din("cpw2", [H, 3])
    cpb2_d = din("cpb2", [3, 1], F32)
    sel_d = din("sel", [N, R])              # per-core one-hot row selector
    onesr_d = din("onesr", [1, 128])        # ones row (K=1 broadcast matmuls)
    onesc_d = din("onesc", [128, 1])        # ones col (LN sums lhsT)
    vone_d = din("vone", [1, 1])            # scalar 1.0 (v_aug ones columns)

    distT_o = nc.dram_tensor("distT", [128, 4, R], F32, kind="ExternalOutput")
    xT_o = nc.dram_tensor("xT", [3, N], F32, kind="ExternalOutput")

    with tile.TileContext(nc) as tc, nc.allow_low_precision(reason="fp16 matmul pipeline"):
        wp = tc.alloc_tile_pool(name="wp", bufs=1)            # persistent weights/state
        ap = tc.alloc_tile_pool(name="ap", bufs=2)            # activations
        wl = tc.alloc_tile_pool(name="wl", bufs=2)            # streamed layer weights
        ps = tc.alloc_tile_pool(name="ps", bufs=1, space="PSUM")

        def wtile(name, shape, dt=F16):
            return wp.tile(shape, dt, tag=name, name=name)

        def ld(t, src):
            nc.sync.dma_start(out=t, in_=src)

        # ---- persistent loads ----
        z0 = wtile("z0", [128, N]); ld(z0, z_inT_d[0:128, :])
        z1 = wtile("z1", [64, N]); ld(z1, z_inT_d[128:ZIN, :])
        zg = wtile("zg", [LAT, 2]); ld(zg, zg_d[:, :])
        ipw1a = wtile("ipw1a", [128, H]); ld(ipw1a, ipw1_d[0:128, :])
        ipw1b = wtile("ipw1b", [64, H]); ld(ipw1b, ipw1_d[128:ZIN, :])
        ipw2 = [wtile(f"ipw2_{t}", [128, H]) for t in range(2)]
        for t in range(2):
            ld(ipw2[t], ipw2_d[t * 128:(t + 1) * 128, :])
        gpw1 = wtile("gpw1", [LAT, H]); ld(gpw1, gpw1_d[:, :])
        gpw2 = [wtile(f"gpw2_{t}", [128, H]) for t in range(2)]
        for t in range(2):
            ld(gpw2[t], gpw2_d[t * 128:(t + 1) * 128, :])

        def bias_cols(name, src, n):
            t = wp.tile([128, n], F32, tag=name, name=name)
            for m in range(n):
                ld(t[:, m:m + 1], src[m * 128:(m + 1) * 128, :])
            return t

        ipb1 = bias_cols("ipb1", ipb1_d, 2)
        ipb2 = bias_cols("ipb2", ipb2_d, 2)
        gpb1 = bias_cols("gpb1", gpb1_d, 2)
        gpb2 = bias_cols("gpb2", gpb2_d, 2)
        dpb1 = bias_cols("dpb1", dpb1_d, 2)
        dpb2 = bias_cols("dpb2", dpb2_d, 2)
        cpb1 = bias_cols("cpb1", cpb1_d, 2)
        cpb2 = wtile("cpb2", [3, 1], F32); ld(cpb2, cpb2_d[:, :])
        dpb3 = wtile("dpb3", [128, 1], F32); ld(dpb3, dpb3_d[:, :])

        dpw1a = [wtile(f"dpw1a_{t}", [128, H]) for t in range(2)]
        dpw1b = [wtile(f"dpw1b_{t}", [128, H]) for t in range(2)]
        dpw2 = [wtile(f"dpw2_{t}", [128, H]) for t in range(2)]
        cpw1 = [wtile(f"cpw1_{t}", [128, H]) for t in range(2)]
        for t in range(2):
            ld(dpw1a[t], dpw1a_d[t * 128:(t + 1) * 128, :])
            ld(dpw1b[t], dpw1b_d[t * 128:(t + 1) * 128, :])
            ld(dpw2[t], dpw2_d[t * 128:(t + 1) * 128, :])
            ld(cpw1[t], cpw1_d[t * 128:(t + 1) * 128, :])
        cpw2 = [wtile(f"cpw2_{t}", [128, 3]) for t in range(2)]
        for t in range(2):
            ld(cpw2[t], cpw2_d[t * 128:(t + 1) * 128, :])
        w3c = wtile("w3c", [128, 4]); ld(w3c, dpw3_d[:, :])
        sel = wtile("sel", [128, 4, R])
        for c in range(4):
            ld(sel[:, c, :], sel_d[c * 128:(c + 1) * 128, :])
        onesr = wtile("onesr", [1, 128]); ld(onesr, onesr_d[:, :])
        onesc = wtile("onesc", [128, 1]); ld(onesc, onesc_d[:, :])

        # v_aug: [128 tok, 4 chunk, 4*65] with ones in cols h*65+64 (persistent)
        v_aug = wtile("v_aug", [128, 4, 4 * 65])
        _vap = vone_d[:, :]
        vone_bcast = bass.AP(tensor=_vap.tensor, offset=_vap.offset,
                             ap=[[0, 128], [0, 16], [1, 1]])
        nc.gpsimd.dma_start(
            out=v_aug.rearrange("p c (h x) -> p (c h) x", x=65)[:, :, 64:65],
            in_=vone_bcast)

        def mmacc(out_ap, pairs):
            last = len(pairs) - 1
            for i, (l, r) in enumerate(pairs):
                nc.tensor.matmul(out_ap, l, r, start=(i == 0), stop=(i == last))

        # ================= stage A: input/global projections =================
        h1T = ap.tile([128, 2, N], F16, tag="h1T")
        for m in range(2):
            p = ps.tile([128, N], F32, tag="mm", bufs=3, name="psA")
            mc = slice(m * 128, (m + 1) * 128)
            mmacc(p, [(ipw1a[:, mc], z0), (ipw1b[:, mc], z1)])
            nc.scalar.activation(h1T[:, m, :], p, AF.Silu, bias=ipb1[:, m:m + 1])

        g1 = ap.tile([128, 4], F16, tag="g1")
        for m in range(2):
            pg = ps.tile([128, 2], F32, tag="st", bufs=2, name="psg")
            mmacc(pg, [(gpw1[:, m * 128:(m + 1) * 128], zg)])
            nc.scalar.activation(g1[:, 2 * m:2 * m + 2], pg, AF.Silu,
                                 bias=gpb1[:, m:m + 1])
        bcol = ap.tile([128, 2], F32, tag="bcol")
        for m in range(2):
            pg2 = ps.tile([128, 2], F32, tag="st", bufs=2, name="psg2")
            mc = slice(m * 128, (m + 1) * 128)
            mmacc(pg2, [(gpw2[t][:, mc], g1[:, 2 * t:2 * t + 2]) for t in range(2)])
            # bcol = ip_b2 + gp_b2 + g2
            nc.vector.tensor_scalar_add(bcol[:, m:m + 1], pg2[:, 0:1], gpb2[:, m:m + 1])
            nc.vector.tensor_add(bcol[:, m:m + 1], bcol[:, m:m + 1], ipb2[:, m:m + 1])

        hT = ap.tile([128, 2, N], F16, tag="hT")
        for m in range(2):
            p = ps.tile([128, N], F32, tag="mm", bufs=3, name="psA2")
            mc = slice(m * 128, (m + 1) * 128)
            mmacc(p, [(ipw2[t][:, mc], h1T[:, t, :]) for t in range(2)])
            nc.scalar.activation(hT[:, m, :], p, AF.Identity, bias=bcol[:, m:m + 1])

        # ================= stage B: transformer layers =================
        for layer in range(NL):
            wqk = [wl.tile([128, 2 * H], F16, tag=f"wqk{t}", name=f"wqk{t}") for t in range(2)]
            wv = [wl.tile([128, H], F16, tag=f"wv{t}", name=f"wv{t}") for t in range(2)]
            wo = [wl.tile([128, H], F16, tag=f"wo{t}", name=f"wo{t}") for t in range(2)]
            w1 = [wl.tile([128, FF], F16, tag=f"w1{t}", name=f"w1{t}") for t in range(2)]
            w2 = [wl.tile([128, H], F16, tag=f"w2{t}", name=f"w2{t}") for t in range(8)]
            for t in range(2):
                ld(wqk[t], wqk_d[layer, t * 128:(t + 1) * 128, :])
                ld(wv[t], wv_d[layer, t * 128:(t + 1) * 128, :])
                ld(wo[t], wo_d[layer, t * 128:(t + 1) * 128, :])
                ld(w1[t], w1_d[layer, t * 128:(t + 1) * 128, :])
            for t in range(8):
                ld(w2[t], w2_d[layer, t * 128:(t + 1) * 128, :])
            bqk = wl.tile([128, 4], F32, tag="bqk", name="bqk")
            for m in range(4):
                ld(bqk[:, m:m + 1], bqk_d[layer, m * 128:(m + 1) * 128, :])
            bv = wl.tile([1, H], F16, tag="bv", name="bv")
            ld(bv, bv_d[layer, :, :])
            bwo = wl.tile([128, 2], F32, tag="bwo", name="bwo")
            b2f = wl.tile([128, 2], F32, tag="b2f", name="b2f")
            ln1s = wl.tile([128, 2], F32, tag="ln1s", name="ln1s")
            ln1b = wl.tile([128, 2], F32, tag="ln1b", name="ln1b")
            ln2s = wl.tile([128, 2], F32, tag="ln2s", name="ln2s")
            ln2b = wl.tile([128, 2], F32, tag="ln2b", name="ln2b")
            for m in range(2):
                ld(bwo[:, m:m + 1], bwo_d[layer, m * 128:(m + 1) * 128, :])
                ld(b2f[:, m:m + 1], b2_d[layer, m * 128:(m + 1) * 128, :])
                ld(ln1s[:, m:m + 1], ln1s_d[layer, m * 128:(m + 1) * 128, :])
                ld(ln1b[:, m:m + 1], ln1b_d[layer, m * 128:(m + 1) * 128, :])
                ld(ln2s[:, m:m + 1], ln2s_d[layer, m * 128:(m + 1) * 128, :])
                ld(ln2b[:, m:m + 1], ln2b_d[layer, m * 128:(m + 1) * 128, :])
            b1f = wl.tile([128, 8], F32, tag="b1f", name="b1f")
            for m in range(8):
                ld(b1f[:, m:m + 1], b1_d[layer, m * 128:(m + 1) * 128, :])

            # --- q,k projections (feature-major) ---
            qkT = ap.tile([128, 4, N], F16, tag="qkT")
            for m in range(4):
                p = ps.tile([128, N], F32, tag="mm", bufs=3, name="psqk")
                mc = slice(m * 128, (m + 1) * 128)
                mmacc(p, [(wqk[t][:, mc], hT[:, t, :]) for t in range(2)])
                nc.scalar.activation(qkT[:, m, :], p, AF.Identity, bias=bqk[:, m:m + 1])

            # --- v (token-major, with +bv via K=1 ones-row matmul) ---
            for c in range(4):
                pv = ps.tile([128, H], F32, tag="mm", bufs=3, name="psv")
                cc = slice(c * 128, (c + 1) * 128)
                pairs = [(hT[:, t, cc], wv[t]) for t in range(2)] + [(onesr, bv)]
                mmacc(pv, pairs)
                nc.vector.tensor_copy(
                    v_aug.rearrange("p c (h x) -> p c h x", x=65)[:, c, :, 0:64],
                    pv.rearrange("p (h x) -> p h x", x=64))

            # --- attention: scoresT -> exp -> o (with ones-col Z) ---
            oT = ap.tile([128, 2, N], F16, tag="oT")
            for h in range(NH):
                rows = slice((h % 2) * 64, (h % 2) * 64 + 64)
                expT = ap.tile([128, 4, N], F16, tag="expT")
                for kc in range(4):
                    psc = ps.tile([128, N], F32, tag="mm", bufs=3, name="pssc")
                    nc.tensor.matmul(psc,
                                     qkT[rows, 2 + h // 2, kc * 128:(kc + 1) * 128],
                                     qkT[rows, h // 2, :], start=True, stop=True)
                    nc.scalar.activation(expT[:, kc, :], psc, AF.Exp, scale=0.125)
                pso = ps.tile([65, N], F32, tag="o", bufs=2, name="pso")
                mmacc(pso, [(v_aug[:, kc, h * 65:(h + 1) * 65], expT[:, kc, :])
                            for kc in range(4)])
                rz = ap.tile([1, N], F16, tag="rz")
                nc.vector.reciprocal(rz, pso[64:65, :])
                psz = ps.tile([128, N], F32, tag="mm", bufs=3, name="psz")
                nc.tensor.matmul(psz, onesr, rz, start=True, stop=True)
                zb = ap.tile([128, N], F32, tag="zb")
                nc.scalar.activation(zb, psz, AF.Copy)
                nc.vector.tensor_mul(oT[(h % 2) * 64:(h % 2) * 64 + 64, h // 2, :],
                                     pso[0:64, :], zb[0:64, :])

            # --- wo + residual + LN1 ---
            res = ap.tile([128, 2, N], F16, tag="res")
            for m in range(2):
                pw = ps.tile([128, N], F32, tag="mm", bufs=3, name="psw")
                mc = slice(m * 128, (m + 1) * 128)
                mmacc(pw, [(wo[t][:, mc], oT[:, t, :]) for t in range(2)])
                t1 = ap.tile([128, N], F16, tag="t1")
                nc.vector.tensor_scalar_add(t1, pw, bwo[:, m:m + 1])
                nc.vector.tensor_add(res[:, m, :], t1, hT[:, m, :])

            def layernorm(src, s_col, b_col, tagpfx):
                sq = ap.tile([128, 2, N], F16, tag="sq")
                nc.scalar.activation(sq, src, AF.Square)
                ps_sum = ps.tile([1, N], F32, tag="st", bufs=2, name="ps_sum")
                mmacc(ps_sum, [(onesc, src[:, t, :]) for t in range(2)])
                ps_sq = ps.tile([1, N], F32, tag="st", bufs=2, name="ps_sq")
                mmacc(ps_sq, [(onesc, sq[:, t, :]) for t in range(2)])
                nmean = ap.tile([1, N], F32, tag="nmean")
                nc.scalar.activation(nmean, ps_sum, AF.Identity, scale=-1.0 / H)
                m2 = ap.tile([1, N], F32, tag="m2")
                nc.scalar.activation(m2, ps_sum, AF.Square, scale=1.0 / H)
                vv = ap.tile([1, N], F32, tag="vv")
                nc.vector.tensor_scalar(vv, ps_sq, 1.0 / H, LN_EPS,
                                        op0=OP.mult, op1=OP.add)
                nc.vector.tensor_sub(vv, vv, m2)
                lnv = ap.tile([1, N], F32, tag="lnv")
                nc.scalar.activation(lnv, vv, AF.Ln)
                inv = ap.tile([1, N], F16, tag="inv")
                nc.scalar.activation(inv, lnv, AF.Exp, scale=-0.5)
                Bn = ap.tile([1, N], F16, tag="Bn")
                nc.vector.tensor_mul(Bn, nmean, inv)
                psA = ps.tile([128, N], F32, tag="mm", bufs=3, name="psA")
                nc.tensor.matmul(psA, onesr, inv, start=True, stop=True)
                psB = ps.tile([128, N], F32, tag="mm", bufs=3, name="psB")
                nc.tensor.matmul(psB, onesr, Bn, start=True, stop=True)
                out = ap.tile([128, 2, N], F16, tag="hT")
                for m in range(2):
                    ta = ap.tile([128, N], F16, tag="t2")
                    nc.vector.tensor_mul(ta, src[:, m, :], psA)
                    tb = ap.tile([128, N], F16, tag="t3")
                    nc.vector.tensor_add(tb, ta, psB)
                    nc.scalar.activation(out[:, m, :], tb, AF.Identity,
                                         bias=b_col[:, m:m + 1], scale=s_col[:, m:m + 1])
                return out

            hT = layernorm(res, ln1s, ln1b, "ln1")

            # --- FFN ---
            f1 = ap.tile([128, 8, N], F16, tag="f1")
            for mo in range(8):
                pf = ps.tile([128, N], F32, tag="mm", bufs=3, name="psf")
                mc = slice(mo * 128, (mo + 1) * 128)
                mmacc(pf, [(w1[t][:, mc], hT[:, t, :]) for t in range(2)])
                nc.vector.tensor_scalar(f1[:, mo, :], pf, b1f[:, mo:mo + 1], 0.0,
                                        op0=OP.add, op1=OP.max)
            res2 = ap.tile([128, 2, N], F16, tag="res")
            for m in range(2):
                pf2 = ps.tile([128, N], F32, tag="mm", bufs=3, name="psf2")
                mc = slice(m * 128, (m + 1) * 128)
                mmacc(pf2, [(w2[t][:, mc], f1[:, t, :]) for t in range(8)])
                t1 = ap.tile([128, N], F16, tag="t1")
                nc.vector.tensor_scalar_add(t1, pf2, b2f[:, m:m + 1])
                nc.vector.tensor_add(res2[:, m, :], t1, hT[:, m, :])
            hT = layernorm(res2, ln2s, ln2b, "ln2")

        # ================= stage C: heads =================
        # coord head
        c1 = ap.tile([128, 2, N], F16, tag="c1")
        for m in range(2):
            p = ps.tile([128, N], F32, tag="mm", bufs=3, name="psc1")
            mc = slice(m * 128, (m + 1) * 128)
            mmacc(p, [(cpw1[t][:, mc], hT[:, t, :]) for t in range(2)])
            nc.scalar.activation(c1[:, m, :], p, AF.Silu, bias=cpb1[:, m:m + 1])
        px = ps.tile([3, N], F32, tag="st", bufs=2, name="psx")
        mmacc(px, [(cpw2[t], c1[:, t, :]) for t in range(2)])
        x_sb = ap.tile([3, N], F32, tag="x_sb")
        nc.scalar.activation(x_sb, px, AF.Identity, bias=cpb2[:, 0:1])
        nc.sync.dma_start(out=xT_o[:, :], in_=x_sb)

        # ajT (feature-major)
        ajT = wtile("ajT", [128, 2, N])
        for m in range(2):
            p = ps.tile([128, N], F32, tag="mm", bufs=3, name="psaj")
            mc = slice(m * 128, (m + 1) * 128)
            mmacc(p, [(dpw1b[t][:, mc], hT[:, t, :]) for t in range(2)])
            nc.scalar.activation(ajT[:, m, :], p, AF.Copy)

        # ai token-major, then select this core's rows and add b1
        ai_tok = ap.tile([128, 4, H], F16, tag="ai_tok")
        for c in range(4):
            p = ps.tile([128, H], F32, tag="mm", bufs=3, name="psai")
            cc = slice(c * 128, (c + 1) * 128)
            mmacc(p, [(hT[:, t, cc], dpw1a[t]) for t in range(2)])
            nc.vector.tensor_copy(ai_tok[:, c, :], p)
        aib = wp.tile([128, 2, R], F32, tag="aib", name="aib")
        for m in range(2):
            p = ps.tile([128, R], F32, tag="st", bufs=2, name="psaib")
            mc = slice(m * 128, (m + 1) * 128)
            mmacc(p, [(ai_tok[:, c, mc], sel[:, c, :]) for c in range(4)])
            nc.scalar.activation(aib[:, m, :], p, AF.Identity, bias=dpb1[:, m:m + 1])

        distT = wtile("distT_sb", [128, 4, R])  # fp16 pre-softplus collect
        ps.release()
        ps2 = tc.alloc_tile_pool(name="ps2", bufs=1, space="PSUM")
        dp = tc.alloc_tile_pool(name="dp", bufs=3)

        # ---- dist head i-loop: groups of 4 rows ----
        for g in range(R // 4):
            pre = dp.tile([128, 2, 4, N], F16, tag="pre", bufs=3)
            for t in range(2):
                for ii in range(4):
                    eng = nc.vector if (ii % 2 == 0) else nc.gpsimd
                    eng.tensor_scalar_add(pre[:, t, ii, :], ajT[:, t, :],
                                          aib[:, t, 4 * g + ii:4 * g + ii + 1])
                nc.scalar.activation(pre[:, t], pre[:, t], AF.Silu)
            for sg in range(2):
                p2sb = dp.tile([128, 2, 2, N], F16, tag="p2sb", bufs=2)
                for mo in range(2):
                    p2 = ps2.tile([128, 2, N], F32, tag="p2", bufs=3, name="p2")
                    mc = slice(mo * 128, (mo + 1) * 128)
                    for ii in range(2):
                        mmacc(p2[:, ii, :],
                              [(dpw2[t][:, mc], pre[:, t, sg * 2 + ii, :])
                               for t in range(2)])
                    nc.scalar.activation(p2sb[:, mo], p2, AF.Silu,
                                         bias=dpb2[:, mo:mo + 1])
                for ii in range(2):
                    i_loc = g * 4 + sg * 2 + ii
                    pdc = ps2.tile([128, 8], F32, tag="pdc", bufs=2, name="pdc")
                    for jc in range(4):
                        mmacc(pdc[:, 2 * jc:2 * jc + 2],
                              [(p2sb[:, mo, ii, jc * 128:(jc + 1) * 128],
                                w3c[:, 2 * mo:2 * mo + 2]) for mo in range(2)])
                    nc.vector.tensor_copy(
                        distT[:, :, i_loc:i_loc + 1],
                        pdc.rearrange("p (a b) -> p a b", b=2)[:, :, 0:1])

        # softplus(x + b3) = Ln(1 + Exp(x + b3))
        dout = dp.tile([128, 4, R], F32, tag="dout", bufs=1)
        nc.scalar.activation(dout, distT, AF.Exp, bias=dpb3[:, 0:1])
        nc.vector.tensor_scalar_add(dout, dout, 1.0)
        nc.scalar.activation(dout, dout, AF.Ln)
        nc.sync.dma_start(out=distT_o[:, :, :], in_=dout)

        dp.release()
        ps2.release()
        wl.release()
        ap.release()
        wp.release()
    nc.compile()
    return nc


def _prep_inputs(inputs):
    inp = {k: np.asarray(v) for k, v in inputs.items()}
    f16 = np.float16
    f32 = np.float32

    z = inp["z"][0].astype(f32)                       # (512, 128)
    at = inp["atom_types"][0].astype(np.int64)        # (512,)
    emb = inp["emb"].astype(f32)                      # (4, 64)
    z_in = np.concatenate([z, emb[at]], axis=1)       # (512, 192)

    common = {
        "z_inT": np.ascontiguousarray(z_in.T).astype(f16),
        "zg": np.ascontiguousarray(
            np.repeat(inp["z_global"].astype(f32).T, 2, axis=1)).astype(f16),
        "ipw1": np.ascontiguousarray(inp["ip_w1"].T).astype(f16),
        "ipb1": inp["ip_b1"].reshape(-1, 1).astype(f32),
        "ipw2": np.ascontiguousarray(inp["ip_w2"].T).astype(f16),
        "ipb2": inp["ip_b2"].reshape(-1, 1).astype(f32),
        "gpw1": np.ascontiguousarray(inp["gp_w1"].T).astype(f16),
        "gpb1": inp["gp_b1"].reshape(-1, 1).astype(f32),
        "gpw2": np.ascontiguousarray(inp["gp_w2"].T).astype(f16),
        "gpb2": inp["gp_b2"].reshape(-1, 1).astype(f32),
        "wqk": np.ascontiguousarray(
            inp["tf_wqkv"].transpose(0, 2, 1)[:, :, 0:2 * H]).astype(f16),
        "bqk": inp["tf_bqkv"][:, 0:2 * H].reshape(NL, 2 * H, 1).astype(f32),
        "wv": np.ascontiguousarray(
            inp["tf_wqkv"].transpose(0, 2, 1)[:, :, 2 * H:3 * H]).astype(f16),
        "bv": inp["tf_bqkv"][:, 2 * H:3 * H].reshape(NL, 1, H).astype(f16),
        "wo": np.ascontiguousarray(inp["tf_wo"].transpose(0, 2, 1)).astype(f16),
        "bwo": inp["tf_bo"].reshape(NL, H, 1).astype(f32),
        "ln1s": inp["tf_ln1_s"].reshape(NL, H, 1).astype(f32),
        "ln1b": inp["tf_ln1_b"].reshape(NL, H, 1).astype(f32),
        "w1": np.ascontiguousarray(inp["tf_w1"].transpose(0, 2, 1)).astype(f16),
        "b1": inp["tf_b1"].reshape(NL, FF, 1).astype(f32),
        "w2": np.ascontiguousarray(inp["tf_w2"].transpose(0, 2, 1)).astype(f16),
        "b2": inp["tf_b2"].reshape(NL, H, 1).astype(f32),
        "ln2s": inp["tf_ln2_s"].reshape(NL, H, 1).astype(f32),
        "ln2b": inp["tf_ln2_b"].reshape(NL, H, 1).astype(f32),
        "dpw1a": np.ascontiguousarray(inp["dp_w1"][:, 0:H].T).astype(f16),
        "dpw1b": np.ascontiguousarray(inp["dp_w1"][:, H:2 * H].T).astype(f16),
        "dpb1": inp["dp_b1"].reshape(-1, 1).astype(f32),
        "dpw2": np.ascontiguousarray(inp["dp_w2"].T).astype(f16),
        "dpb2": inp["dp_b2"].reshape(-1, 1).astype(f32),
        "dpw3": np.ascontiguousarray(
            np.stack([inp["dp_w3"][0, 0:128], inp["dp_w3"][0, 0:128],
                      inp["dp_w3"][0, 128:256], inp["dp_w3"][0, 128:256]],
                     axis=1)).astype(f16),
        "dpb3": np.full((128, 1), float(inp["dp_b3"][0]), f32),
        "cpw1": np.ascontiguousarray(inp["cp_w1"].T).astype(f16),
        "cpb1": inp["cp_b1"].reshape(-1, 1).astype(f32),
        "cpw2": np.ascontiguousarray(inp["cp_w2"].T).astype(f16),
        "cpb2": inp["cp_b2"].reshape(-1, 1).astype(f32),
        "onesr": np.ones((1, 128), f16),
        "onesc": np.ones((128, 1), f16),
        "vone": np.ones((1, 1), f16),
    }
    in_maps = []
    for k in range(NCORES):
        sel = np.zeros((N, R), f16)
        sel[np.arange(R) + k * R, np.arange(R)] = 1.0
        m = dict(common)
        m["sel"] = sel
        in_maps.append(m)
    return in_maps


def kernel(**inputs):
    if "nc" not in _BUILD_CACHE:
        _BUILD_CACHE["nc"] = _build()
    nc = _BUILD_CACHE["nc"]
    in_maps = _prep_inputs(inputs)
    res = run_bass_kernel_spmd(nc, in_maps, core_ids=list(range(NCORES)))
    rows = []
    for k in range(NCORES):
        dT = res.results[k]["distT"]                   # [128, 4, 64]
        rows.append(dT.transpose(2, 1, 0).reshape(R, N))
    dist = np.concatenate(rows, axis=0)                # (512, 512)
    dist = (0.5 * (dist + dist.T)).astype(np.float32)
    x = res.results[0]["xT"].T.astype(np.float32)      # (512, 3)
    return x[None], dist[None]
